# revision 14
# baseline (speedup 1.0000x reference)
"""Trainium2 Bass kernel for nn_MultiHeadAttention (B=2, S=2048, E=1024, H=16).

Sharding: 8 cores = data-parallel over batch (2) x tensor-parallel over head
groups (4 heads/core). Each core computes its head group's QKV projection,
attention, and a partial output projection; the host sums the 4 partials per
batch and adds the output bias.

The reference mask adds -1e9 to the lower triangle INCLUDING the diagonal, so
query q attends only to keys k > q, except the last row (all keys masked) which
degenerates to a plain softmax over all keys == uniform weights (because
-1e9 + s rounds to exactly -1e9 in fp32, so after max-subtraction every entry
is 0). The device kernel produces NaN for that row (0/0); the host patches it
analytically: out[S-1] = mean_s(v[s]) @ Wout^T + bout.

Device dataflow per core (all matmuls fp32r, 1 cycle/row at N>=512):
  x [2048,1024] --PE transpose--> xT [1024,2048]
  qkT = WqkT^T. xT   (q,k in [dim, seq] layout, heads packed 2/partition-tile)
  v   = xT^T . WvT   (natural [seq, dim] layout + bias, ones column appended)
  scoresT[sk,sq] = k qT ; exp via ACT (scale=1/8); anti-causal mask via
  affine_select on diagonal tiles; fully-masked tiles skipped entirely.
  valuesT'[d',sq] = v'^T expT  accumulated over sk tiles; row 64 = softmax
  denominator (ones column trick). Normalization: reciprocal + PE broadcast
  (indicator matmul) + elementwise multiply. Partial out = vcat^T WoutT.
"""

import numpy as np
from contextlib import ExitStack

B, S, E, H = 2, 2048, 1024, 16
HD = 64          # head dim
HPC = 4          # heads per core
F = HPC * HD     # 256: local feature dim
NCORES = 8

_compiled = {}


def _build_nc():
    import concourse.bacc as bacc
    import concourse.bass as bass
    import concourse.mybir as mybir
    import concourse.tile as tile
    from concourse.masks import make_identity

    f32 = mybir.dt.float32
    f32r = mybir.dt.float32r
    AF = mybir.ActivationFunctionType
    OP = mybir.AluOpType

    nc = bacc.Bacc(None, target_bir_lowering=False)

    x_d = nc.dram_tensor("x", [S, E], f32, kind="ExternalInput")
    wqk_d = nc.dram_tensor("wqk", [E, 512], f32r, kind="ExternalInput")
    wv_d = nc.dram_tensor("wv", [E, F], f32r, kind="ExternalInput")
    wout_d = nc.dram_tensor("wout", [F, E], f32r, kind="ExternalInput")
    bqk_d = nc.dram_tensor("bqk", [128, 4], f32, kind="ExternalInput")
    bvb_d = nc.dram_tensor("bvb", [128, F], f32, kind="ExternalInput")
    ind_d = nc.dram_tensor("ind", [34, 128], f32r, kind="ExternalInput")
    vones_d = nc.dram_tensor("vones", [128, 64], f32r, kind="ExternalInput")
    out_d = nc.dram_tensor("out", [S, E], f32, kind="ExternalOutput")

    NST = S // 128        # 16 seq tiles of 128
    NSC = S // 512        # 4 seq chunks of 512
    NET = E // 128        # 8 embed tiles

    with tile.TileContext(nc) as tc:
        with ExitStack() as ctx:
            const = ctx.enter_context(tc.tile_pool(name="const", bufs=1))
            ident = const.tile([128, 128], f32)
            make_identity(nc, ident[:])

            indsb = const.tile([34, 128], f32r)
            nc.sync.dma_start(indsb[:], ind_d[:, :])

            wqk = const.tile([128, NET, 512], f32r)
            nc.sync.dma_start(wqk[:], wqk_d.ap().rearrange("(kt p) m -> p kt m", p=128))
            wv = const.tile([128, NET, F], f32r)
            nc.sync.dma_start(wv[:], wv_d.ap().rearrange("(kt p) m -> p kt m", p=128))
            wout = const.tile([128, 2, E], f32r)
            nc.sync.dma_start(wout[:], wout_d.ap().rearrange("(ft p) e -> p ft e", p=128))
            bqk = const.tile([128, 4], f32)
            nc.sync.dma_start(bqk[:], bqk_d[:, :])
            bvb = const.tile([128, HPC, HD], f32)
            nc.sync.dma_start(bvb[:], bvb_d.ap().rearrange("p (h d) -> p h d", d=HD))

            qsb = const.tile([128, 2, S], f32r)
            ksb = const.tile([128, 2, S], f32r)
            vsb = const.tile([128, NST, HPC, HD + 1], f32r)
            # ones column (softmax-denominator trick) shipped from host:
            # memset doesn't support float32r
            nc.sync.dma_start(vsb[:, :, :, HD:HD + 1], vones_d.ap().rearrange(
                "p (a b c) -> p a b c", b=HPC, c=1))
            vcat = const.tile([128, 2, S], f32r)
            denomsb = const.tile([34, S], f32)
            recipsb = const.tile([34, S], f32r)

            # ---------------- Phase A: transpose x, project q/k/v ----------
            with ExitStack() as ctxA:
                xnat = ctxA.enter_context(tc.tile_pool(name="xnat", bufs=5))
                xTp = ctxA.enter_context(tc.tile_pool(name="xTp", bufs=2))
                psA = ctxA.enter_context(tc.tile_pool(name="psA", bufs=2, space="PSUM"))
                psT = ctxA.enter_context(tc.tile_pool(name="psT", bufs=4, space="PSUM"))

                xT_tiles = [None] * NSC

                def emit_transpose(sc):
                    xTt = xTp.tile([128, NET, 512], f32r, tag="xTt")
                    xT_tiles[sc] = xTt
                    for st4 in range(4):
                        stile = sc * 4 + st4
                        xn = xnat.tile([128, E], f32, tag="xn")
                        nc.sync.dma_start(xn[:], x_d[stile * 128:(stile + 1) * 128, :])
                        for et in range(NET):
                            ptr = psT.tile([128, 128], f32, tag="ptr")
                            nc.tensor.transpose(ptr[:], xn[:, et * 128:(et + 1) * 128], ident[:])
                            nc.vector.tensor_copy(xTt[:, et, st4 * 128:(st4 + 1) * 128], ptr[:])

                def emit_proj(sc):
                    xTt = xT_tiles[sc]
                    # q/k projection: out m-tiles [q01, q23, k01, k23], n = this chunk
                    for mt in range(4):
                        pqk = psA.tile([128, 512], f32, tag="pqk")
                        for kt in range(NET):
                            nc.tensor.matmul(
                                pqk[:],
                                wqk[:, kt, mt * 128:(mt + 1) * 128],
                                xTt[:, kt, :],
                                start=(kt == 0), stop=(kt == NET - 1),
                            )
                        dst = qsb if mt < 2 else ksb
                        nc.vector.tensor_scalar_add(
                            dst[:, mt % 2, sc * 512:(sc + 1) * 512], pqk[:], bqk[:, mt:mt + 1]
                        )
                    # v projection (natural layout): m = seq tile, n = 256
                    for st4 in range(4):
                        stile = sc * 4 + st4
                        pv = psA.tile([128, F], f32, tag="pv")
                        for kt in range(NET):
                            nc.tensor.matmul(
                                pv[:],
                                xTt[:, kt, st4 * 128:(st4 + 1) * 128],
                                wv[:, kt, :],
                                start=(kt == 0), stop=(kt == NET - 1),
                            )
                        nc.vector.tensor_tensor(
                            out=vsb[:, stile, :, 0:HD],
                            in0=pv[:].rearrange("p (h d) -> p h d", d=HD),
                            in1=bvb[:],
                            op=OP.add,
                        )

                for sc in range(NSC):
                    emit_transpose(sc)
                    if sc >= 1:
                        emit_proj(sc - 1)
                emit_proj(NSC - 1)

            # ---------------- Phase B: attention + output projection -------
            with ExitStack() as ctxB:
                expp = ctxB.enter_context(tc.tile_pool(name="expp", bufs=4))
                stgp = ctxB.enter_context(tc.tile_pool(name="stgp", bufs=3))
                outp = ctxB.enter_context(tc.tile_pool(name="outp", bufs=3))
                psS = ctxB.enter_context(tc.tile_pool(name="psS", bufs=3, space="PSUM"))
                psV = ctxB.enter_context(tc.tile_pool(name="psV", bufs=2, space="PSUM"))
                psO = ctxB.enter_context(tc.tile_pool(name="psO", bufs=2, space="PSUM"))

                # flat unit list: (cp, h, t), t = sk tile, anti-causal skip
                units = []
                for cp in range(NSC):
                    for h in range(HPC):
                        for t in range(4 * cp, NST):
                            units.append((cp, h, t))

                exp_tiles = {}
                vpsum = {}

                def emit_S(u):
                    cp, h, t = u
                    base = 64 * (h % 2)
                    hp = h // 2
                    ps = psS.tile([128, 512], f32, tag="ps")
                    nc.tensor.matmul(
                        ps[:],
                        ksb[base:base + 64, hp, t * 128:(t + 1) * 128],
                        qsb[base:base + 64, hp, cp * 512:(cp + 1) * 512],
                    )
                    ex = expp.tile([128, 512], f32r, tag="ex")
                    nc.scalar.activation(ex[:], ps[:], AF.Exp, scale=0.125)
                    r = t - 4 * cp
                    if 0 <= r < 4:
                        # keep iff global sk > global sq: 128r + p - j > 0
                        nc.gpsimd.affine_select(
                            out=ex[:], in_=ex[:], pattern=[[-1, 512]],
                            compare_op=OP.is_gt, fill=0.0,
                            base=128 * r, channel_multiplier=1,
                        )
                    exp_tiles[u] = ex

                def emit_V(u):
                    cp, h, t = u
                    ex = exp_tiles.pop(u)
                    if t == 4 * cp:
                        vpsum[(cp, h)] = psV.tile([HD + 1, 512], f32, tag="pvals", name="pvals")
                    pvals = vpsum[(cp, h)]
                    nc.tensor.matmul(
                        pvals[:],
                        vsb[:, t, h, :],
                        ex[:],
                        start=(t == 4 * cp), stop=(t == NST - 1),
                    )
                    if t == NST - 1:
                        row = 32 * (h // 2) + (h % 2)
                        stg = stgp.tile([HD + 1, 512], f32, tag="stg", name="stg")
                        nc.scalar.activation(stg[:], pvals[:], AF.Copy)
                        nc.sync.dma_start(
                            vcat[64 * (h % 2):64 * (h % 2) + 64, h // 2,
                                 cp * 512:(cp + 1) * 512].bitcast(f32),
                            stg[0:HD, :],
                        )
                        nc.sync.dma_start(
                            denomsb[row:row + 1, cp * 512:(cp + 1) * 512],
                            stg[HD:HD + 1, :],
                        )
                        del vpsum[(cp, h)]

                def emit_norm_and_outproj(cp):
                    for ft in range(2):
                        rb = 32 * ft
                        with nc.allow_low_precision(reason="f32r rounding of softmax reciprocal"):
                            nc.vector.reciprocal(
                                recipsb[rb:rb + 2, cp * 512:(cp + 1) * 512],
                                denomsb[rb:rb + 2, cp * 512:(cp + 1) * 512],
                            )
                        pb = psO.tile([128, 512], f32, tag="po")
                        nc.tensor.matmul(
                            pb[:],
                            indsb[rb:rb + 2, :],
                            recipsb[rb:rb + 2, cp * 512:(cp + 1) * 512],
                        )
                        nc.vector.tensor_tensor(
                            out=vcat[:, ft, cp * 512:(cp + 1) * 512],
                            in0=vcat[:, ft, cp * 512:(cp + 1) * 512].bitcast(f32),
                            in1=pb[:],
                            op=OP.mult,
                        )
                    for st4 in range(4):
                        stile = cp * 4 + st4
                        for nck in range(2):
                            po = psO.tile([128, 512], f32, tag="po")
                            for ft in range(2):
                                nc.tensor.matmul(
                                    po[:],
                                    vcat[:, ft, stile * 128:(stile + 1) * 128],
                                    wout[:, ft, nck * 512:(nck + 1) * 512],
                                    start=(ft == 0), stop=(ft == 1),
                                )
                            osb = outp.tile([128, 512], f32, tag="osb", name="osb")
                            nc.scalar.activation(osb[:], po[:], AF.Copy)
                            nc.sync.dma_start(
                                out_d[stile * 128:(stile + 1) * 128,
                                      nck * 512:(nck + 1) * 512],
                                osb[:],
                            )

                LAG = 2
                done_cp = -1
                for i, u in enumerate(units):
                    emit_S(u)
                    if i >= LAG:
                        emit_V(units[i - LAG])
                        vcp = units[i - LAG][0]
                        if vcp > done_cp and units[i - LAG] == (vcp, HPC - 1, NST - 1):
                            done_cp = vcp
                            emit_norm_and_outproj(vcp)
                for j in range(len(units) - LAG, len(units)):
                    emit_V(units[j])
                    vcp = units[j][0]
                    if vcp > done_cp and units[j] == (vcp, HPC - 1, NST - 1):
                        done_cp = vcp
                        emit_norm_and_outproj(vcp)

    nc.compile()
    return nc


def _get_compiled():
    if "nc" not in _compiled:
        _compiled["nc"] = _build_nc()
    return _compiled["nc"]


def _pack_inputs(x, Wqkv, bqkv, Wout, bout):
    """Per-core input maps. Core c = b*4 + g."""
    in_maps = []
    for b in range(B):
        xb = np.ascontiguousarray(np.asarray(x[b], dtype=np.float32))
        for g in range(HPC):
            heads = [4 * g + lh for lh in range(HPC)]
            qrows = np.concatenate([np.arange(h * 192, h * 192 + 64) for h in heads])
            krows = np.concatenate([np.arange(h * 192 + 64, h * 192 + 128) for h in heads])
            vrows = np.concatenate([np.arange(h * 192 + 128, h * 192 + 192) for h in heads])
            qk = np.concatenate([qrows, krows])
            wqkT = np.ascontiguousarray(Wqkv[qk].T)            # [1024, 512]
            wvT = np.ascontiguousarray(Wqkv[vrows].T)          # [1024, 256]
            woutT = np.ascontiguousarray(Wout[:, 256 * g:256 * (g + 1)].T)  # [256, 1024]
            bqk_p = np.ascontiguousarray(bqkv[qk].reshape(4, 128).T)        # [128, 4]
            bv = bqkv[vrows].astype(np.float32)
            bvb = np.ascontiguousarray(np.broadcast_to(bv[None, :], (128, F)))
            ind = np.zeros((34, 128), dtype=np.float32)
            for rb in (0, 32):
                ind[rb, 0:64] = 1.0
                ind[rb + 1, 64:128] = 1.0
            in_maps.append({
                "x": xb,
                "wqk": wqkT.astype(np.float32),
                "wv": wvT.astype(np.float32),
                "wout": woutT.astype(np.float32),
                "bqk": bqk_p.astype(np.float32),
                "bvb": bvb.astype(np.float32),
                "ind": ind,
                "vones": np.ones((128, 64), dtype=np.float32),
            })
    return in_maps


def _last_row_patch(x, Wqkv, bqkv, Wout, bout):
    """Reference's fully-masked last row == uniform attention over all keys."""
    vrows = np.concatenate(
        [np.arange(h * 192 + 128, h * 192 + 192) for h in range(H)])
    Wv = Wqkv[vrows]              # [1024, 1024], rows in head-major order = E order
    bv = bqkv[vrows]
    out = np.empty((B, E), dtype=np.float32)
    for b in range(B):
        xmean = np.asarray(x[b], dtype=np.float32).mean(axis=0)
        vmean = xmean @ Wv.T + bv
        out[b] = vmean @ Wout.T + bout
    return out


def kernel(x, Wqkv, bqkv, Wout, bout, _results_hook=None):
    from concourse.bass_utils import run_bass_kernel_spmd

    x = np.asarray(x, dtype=np.float32)
    Wqkv = np.asarray(Wqkv, dtype=np.float32)
    bqkv = np.asarray(bqkv, dtype=np.float32)
    Wout = np.asarray(Wout, dtype=np.float32)
    bout = np.asarray(bout, dtype=np.float32)

    nc = _get_compiled()
    in_maps = _pack_inputs(x, Wqkv, bqkv, Wout, bout)
    res = run_bass_kernel_spmd(nc, in_maps, list(range(NCORES)))
    if _results_hook is not None:
        _results_hook(res)

    out = np.empty((B, S, E), dtype=np.float32)
    for b in range(B):
        acc = res.results[4 * b]["out"].astype(np.float32).copy()
        for g in range(1, HPC):
            acc += res.results[4 * b + g]["out"]
        out[b] = acc + bout[None, :]
    out[:, S - 1, :] = _last_row_patch(x, Wqkv, bqkv, Wout, bout)
    return out
